# revision 13
# baseline (speedup 1.0000x reference)
"""Trainium2 Bass kernel for nn_DecoderBlock (B=1, S=2048, D=1024, H=16, DQ=64, DM=4096).

Strategy (8 NeuronCores, one chip):
  - Attention tensor-parallel over heads: core c owns heads {2c, 2c+1}.
    Per-head projections are fused with the QKV projections:
      weff_x[h] = H_x[h] @ W_x  (computed on device), then
      qT/kT = weffT.T-contractions against E.T; v computed in natural [t, e] layout.
    Scores are computed TRANSPOSED ([keys, queries] layout) so that softmax's
    reduction lands on the matmul contraction axis: the row-of-ones trick in the
    o-matmul produces the softmax denominators for free, and no transposes of the
    big score matrix are ever needed.
  - One AllToAll exchanges attention outputs (heads-sharded -> sequence-sharded).
  - Post-attention (W_O, LN1, MLP, LN2, residuals) data-parallel over tokens:
    core c owns tokens [c*256, (c+1)*256), all in transposed [d, s] layout;
    LayerNorm statistics via ones-matmuls over the partition axis.
  - Host-side prep is layout-only: slicing, transposition and bf16 casting of
    inputs; all FLOPs (matmuls, softmax, layernorm) run on device.
"""

import numpy as np
import ml_dtypes

BF16 = ml_dtypes.bfloat16

B, S_FULL, D, H, DQ, DM = 1, 2048, 1024, 16, 64, 4096
NC = 8          # cores
P = 128         # partitions
HPC = H // NC   # heads per core
EPC = HPC * DQ  # per-core attention width (128)
DK = D // P     # d-model chunks (8)
MK = DM // P    # mlp chunks (32)
EPS = 1e-5


def _body(tc, io, S):
    import concourse.bass as bass
    import concourse.mybir as mybir
    from concourse.masks import make_identity

    nc = tc.nc
    fp32 = mybir.dt.float32
    bf16 = mybir.dt.bfloat16
    f32r = mybir.dt.float32r
    Exp = mybir.ActivationFunctionType.Exp
    Relu = mybir.ActivationFunctionType.Relu
    Sqrt = mybir.ActivationFunctionType.Sqrt
    sub_op = mybir.AluOpType.subtract
    mult_op = mybir.AluOpType.mult
    add_op = mybir.AluOpType.add

    SL = S // NC                 # tokens per core
    TT = S // P                  # key tiles
    SPW = min(1024, S)           # score-tile width along queries
    NSP = S // SPW
    W5 = min(512, SPW)           # matmul free-dim chunk
    N5 = SPW // W5
    SLT = min(P, SL)             # partition tile for final transpose
    NST = (SL + P - 1) // P

    ts = bass.ts

    # ---------------- persistent SBUF ----------------
    import contextlib
    _ctx = contextlib.ExitStack()
    persist = _ctx.enter_context(tc.tile_pool(name="persist", bufs=1))

    def ptile(shape, dt, tag):
        return persist.tile(shape, dt, tag=tag, name=tag)

    drampool = _ctx.enter_context(tc.tile_pool(name="drampool", bufs=1, space="DRAM"))

    qt_sb = ptile([EPC, S], f32r, "qt_sb")
    kt_sb = ptile([EPC, S], f32r, "kt_sb")
    ot_sb = ptile([EPC, S], bf16, "ot_sb")            # attn out (2 heads)
    weff_sb = ptile([P, 3, DK, P], f32r, "weff_sb")   # weffT q/k/v
    vaug_sb = ptile([P, HPC, TT, DQ + 1], f32r, "vaug_sb")
    ones_sb = ptile([P, 1], fp32, "ones_sb")
    ident_sb = ptile([P, P], fp32, "ident_sb")
    onesr_sb = ptile([P, 1], f32r, "onesr_sb")
    gb_sb = ptile([P, 6, DK], fp32, "gb_sb")          # g1,b1,g2,b2,l2b,(pad)
    l1b_sb = ptile([P, MK], fp32, "l1b_sb")
    x1t_sb = ptile([P, DK, SL], fp32, "x1t_sb")
    x1tb_sb = ptile([P, DK, SL], f32r, "x1tb_sb")
    att_sb = ptile([P, DK, SL], fp32, "att_sb")
    mlp_sb = ptile([P, DK, SL], fp32, "mlp_sb")
    ots_sb = ptile([P, DK, SL], bf16, "ots_sb")       # o for my tokens
    wot_sb = ptile([P, DK, D], bf16, "wot_sb")

    nc.gpsimd.memset(ones_sb[:], 1.0)
    nc.vector.tensor_copy(onesr_sb[:], ones_sb[:])
    make_identity(nc, ident_sb[:])
    for i, name in enumerate(("g1", "b1", "g2", "b2", "l2b")):
        nc.sync.dma_start(gb_sb[:, i, :], io[name].rearrange("(g p) -> p g", p=P))
    nc.sync.dma_start(l1b_sb[:], io["l1b"].rearrange("(g p) -> p g", p=P))

    # ---------------- weffT = W.T @ H.T ----------------
    etctx = contextlib.ExitStack()
    etpool = etctx.enter_context(tc.tile_pool(name="etpool", bufs=1))
    et_sb = etpool.tile([P, DK, S], f32r, tag="et_sb", name="et_sb")
    for g in range(DK):
        nc.sync.dma_start(et_sb[:, g, :], io["eT"][ts(g, P), :])

    with tc.tile_pool(name="wpool", bufs=2) as wpool, \
         tc.tile_pool(name="wpsum", bufs=2, space="PSUM") as wpsum:
        for j, (wn, hn) in enumerate((("wq", "htq"), ("wk", "htk"), ("wv", "htv"))):
            ht = wpool.tile([P, DK, EPC], f32r, tag="ht")
            for kk in range(DK):
                nc.sync.dma_start(ht[:, kk, :], io[hn][ts(kk, P), :])
            for m in range(DK):
                wt = wpool.tile([P, DK, P], f32r, tag="wt")
                nc.sync.dma_start(
                    wt[:], io[wn][:, ts(m, P)].rearrange("(kk p) e -> p kk e", p=P))
                ps = wpsum.tile([P, EPC], fp32, tag="wps")
                for kk in range(DK):
                    nc.tensor.matmul(ps[:], wt[:, kk, :], ht[:, kk, :],
                                     start=(kk == 0), stop=(kk == DK - 1))
                if j == 0:  # fold 1/sqrt(DQ) into q weights
                    nc.vector.tensor_scalar_mul(weff_sb[:, j, m, :], ps[:], 1.0 / np.sqrt(DQ))
                else:
                    nc.vector.tensor_copy(weff_sb[:, j, m, :], ps[:])

    # ---------------- qT, kT, v ----------------
    with tc.tile_pool(name="qkpsum", bufs=3, space="PSUM") as qkpsum:
        for j, dst in ((0, qt_sb), (1, kt_sb)):
            for sp in range(S // 512):
                ps = qkpsum.tile([EPC, 512], fp32, tag="qkps")
                for kk in range(DK):
                    nc.tensor.matmul(ps[:], weff_sb[:, j, kk, :], et_sb[:, kk, ts(sp, 512)],
                                     start=(kk == 0), stop=(kk == DK - 1))
                nc.vector.tensor_copy(dst[:, ts(sp, 512)], ps[:])
        for t in range(TT):
            ps = qkpsum.tile([P, EPC], fp32, tag="vps")
            for kk in range(DK):
                nc.tensor.matmul(ps[:], et_sb[:, kk, ts(t, P)], weff_sb[:, 2, kk, :],
                                 start=(kk == 0), stop=(kk == DK - 1))
            for h in range(HPC):
                nc.vector.tensor_copy(vaug_sb[:, h, t, :DQ], ps[:, ts(h, DQ)])
                nc.vector.tensor_copy(vaug_sb[:, h, t, DQ:], onesr_sb[:])
    etctx.close()

    # ---------------- attention ----------------
    with tc.tile_pool(name="mkpool", bufs=3) as mkpool, \
         tc.tile_pool(name="expool", bufs=3) as expool, \
         tc.tile_pool(name="scpsum", bufs=2, space="PSUM") as scpsum, \
         tc.tile_pool(name="opsum", bufs=1, space="PSUM") as opsum, \
         tc.tile_pool(name="nrmpool", bufs=2) as nrmpool:
        for sp in range(NSP):
            pso = [opsum.tile([DQ + 1, SPW], fp32, tag=f"pso{h}", name=f"pso{h}")
                   for h in range(HPC)]
            for t in range(TT):
                mk = mkpool.tile([P, SPW], bf16, tag="mk")
                nc.sync.dma_start(mk[:], io["maskT"][ts(t, P), ts(sp, SPW)])
                for h in range(HPC):
                    ps = scpsum.tile([P, SPW], fp32, tag="scps")
                    hs = slice(h * DQ, (h + 1) * DQ)
                    for n in range(N5):
                        nc.tensor.matmul(
                            ps[:, ts(n, W5)], kt_sb[hs, ts(t, P)],
                            qt_sb[hs, sp * SPW + n * W5:sp * SPW + (n + 1) * W5],
                            start=True, stop=True)
                    ex = expool.tile([P, SPW], f32r, tag="ex")
                    nc.scalar.activation(ex[:], ps[:], Exp)
                    nc.vector.tensor_mul(ex[:], ex[:], mk[:])
                    for n in range(N5):
                        nc.tensor.matmul(pso[h][:, ts(n, W5)], vaug_sb[:, h, t, :],
                                         ex[:, ts(n, W5)],
                                         start=(t == 0), stop=(t == TT - 1))
            for h in range(HPC):
                rc = nrmpool.tile([1, SPW], fp32, tag="rc")
                nc.vector.reciprocal(rc[:], pso[h][DQ:DQ + 1, :])
                rcb = nrmpool.tile([DQ, SPW], fp32, tag="rcb")
                nc.gpsimd.partition_broadcast(rcb[:], rc[:], channels=DQ)
                nc.vector.tensor_mul(ot_sb[ts(h, DQ), ts(sp, SPW)], pso[h][:DQ, :],
                                     rcb[:])

    # ---------------- AllToAll: heads-sharded -> token-sharded ----------------
    a2a_in = drampool.tile([NC * EPC, SL], bf16, tag="a2a_in", name="a2a_in")
    a2a_out = drampool.tile([NC * EPC, SL], bf16, tag="a2a_out", name="a2a_out")
    for g in range(NC):
        nc.sync.dma_start(a2a_in[ts(g, EPC), :], ot_sb[:, ts(g, SL)])
    nc.gpsimd.collective_compute(
        "AllToAll", mybir.AluOpType.bypass,
        replica_groups=[list(range(NC))],
        ins=[a2a_in.opt()], outs=[a2a_out.opt()])
    for g in range(NC):
        nc.sync.dma_start(ots_sb[:, g, :], a2a_out[ts(g, EPC), :])

    # ---------------- W_O projection (attT) ----------------
    for g in range(DK):
        nc.sync.dma_start(wot_sb[:, g, :], io["wot"][ts(g, P), :])
    with tc.tile_pool(name="atpsum", bufs=2, space="PSUM") as atpsum:
        for m in range(DK):
            ps = atpsum.tile([P, SL], fp32, tag="atps")
            for g in range(DK):
                nc.tensor.matmul(ps[:], wot_sb[:, g, ts(m, P)], ots_sb[:, g, :],
                                 start=(g == 0), stop=(g == DK - 1))
            nc.vector.tensor_copy(att_sb[:, m, :], ps[:])

    # ---------------- LayerNorm helper (stats over partition axis) -------------
    def layer_norm_T(src_sb, dst_sb, gi, bi, res_sb, pools):
        """dst = res + gamma*(src-mean)/sqrt(var+eps) + beta, all [P, DK, SL] f32."""
        sqpool, stpsum, stpool = pools
        ps1 = stpsum.tile([1, SL], fp32, tag="ps1")
        ps2 = stpsum.tile([1, SL], fp32, tag="ps2")
        for g in range(DK):
            sq = sqpool.tile([P, SL], fp32, tag="sq")
            nc.vector.tensor_mul(sq[:], src_sb[:, g, :], src_sb[:, g, :])
            nc.tensor.matmul(ps1[:], ones_sb[:], src_sb[:, g, :],
                             start=(g == 0), stop=(g == DK - 1))
            nc.tensor.matmul(ps2[:], ones_sb[:], sq[:],
                             start=(g == 0), stop=(g == DK - 1))
        mean = stpool.tile([1, SL], fp32, tag="mean")
        nc.vector.tensor_scalar_mul(mean[:], ps1[:], 1.0 / D)
        var = stpool.tile([1, SL], fp32, tag="var")
        nc.vector.tensor_scalar_mul(var[:], ps2[:], 1.0 / D)
        msq = stpool.tile([1, SL], fp32, tag="msq")
        nc.vector.tensor_mul(msq[:], mean[:], mean[:])
        nc.vector.tensor_tensor(var[:], var[:], msq[:], sub_op)
        nc.vector.tensor_scalar_add(var[:], var[:], EPS)
        std = stpool.tile([1, SL], fp32, tag="std")
        nc.scalar.activation(std[:], var[:], Sqrt)
        rstd = stpool.tile([1, SL], fp32, tag="rstd")
        nc.vector.reciprocal(rstd[:], std[:])
        mrs = stpool.tile([1, SL], fp32, tag="mrs")  # mean*rstd
        nc.vector.tensor_mul(mrs[:], mean[:], rstd[:])
        rstdb = stpool.tile([P, SL], fp32, tag="rstdb")
        nc.gpsimd.partition_broadcast(rstdb[:], rstd[:])
        mrsb = stpool.tile([P, SL], fp32, tag="mrsb")
        nc.gpsimd.partition_broadcast(mrsb[:], mrs[:])
        for g in range(DK):
            t1 = sqpool.tile([P, SL], fp32, tag="t1")
            nc.vector.tensor_mul(t1[:], src_sb[:, g, :], rstdb[:])
            nc.vector.tensor_tensor(t1[:], t1[:], mrsb[:], sub_op)
            nc.vector.tensor_scalar(t1[:], t1[:], gb_sb[:, gi, g:g + 1],
                                    gb_sb[:, bi, g:g + 1], mult_op, add_op)
            nc.vector.tensor_tensor(dst_sb[:, g, :], t1[:], res_sb[:, g, :], add_op)

    with tc.tile_pool(name="sqpool", bufs=3) as sqpool, \
         tc.tile_pool(name="stpsum", bufs=2, space="PSUM") as stpsum, \
         tc.tile_pool(name="stpool", bufs=2) as stpool:
        eres = sqpool.tile([P, DK, SL], fp32, tag="eres")
        for g in range(DK):
            nc.sync.dma_start(eres[:, g, :], io["eresT"][ts(g, P), :])
        layer_norm_T(att_sb, x1t_sb, 0, 1, eres, (sqpool, stpsum, stpool))
        for g in range(DK):
            nc.vector.tensor_copy(x1tb_sb[:, g, :], x1t_sb[:, g, :])

    # ---------------- MLP ----------------
    with tc.tile_pool(name="l1pool", bufs=2) as l1pool, \
         tc.tile_pool(name="hallpool", bufs=1) as hallpool, \
         tc.tile_pool(name="hpsum", bufs=3, space="PSUM") as hpsum:
        ht_all = hallpool.tile([P, MK, SL], f32r, tag="ht_all")
        for mg in range(MK // 4):  # m-chunk groups of 4
            l1t = l1pool.tile([P, DK, 512], f32r, tag="l1t")
            for g in range(DK):
                nc.sync.dma_start(l1t[:, g, :], io["l1t"][ts(g, P), ts(mg, 512)])
            for mi in range(4):
                mc = mg * 4 + mi
                psh = hpsum.tile([P, SL], fp32, tag="psh")
                for g in range(DK):
                    nc.tensor.matmul(psh[:], l1t[:, g, ts(mi, P)], x1tb_sb[:, g, :],
                                     start=(g == 0), stop=(g == DK - 1))
                nc.scalar.activation(ht_all[:, mc, :], psh[:], Relu,
                                     bias=l1b_sb[:, mc:mc + 1])
        with tc.tile_pool(name="l2pool", bufs=2) as l2pool, \
             tc.tile_pool(name="mlppsum", bufs=2, space="PSUM") as mlppsum:
            for dt in range(DK):
                l2td = l2pool.tile([P, MK, P], f32r, tag="l2td")
                nc.sync.dma_start(
                    l2td[:], io["l2t"][:, ts(dt, P)].rearrange("(mc p) d -> p mc d", p=P))
                psm2 = mlppsum.tile([P, SL], fp32, tag="psm2")
                for mc in range(MK):
                    nc.tensor.matmul(psm2[:], l2td[:, mc, :], ht_all[:, mc, :],
                                     start=(mc == 0), stop=(mc == MK - 1))
                nc.vector.tensor_scalar_add(mlp_sb[:, dt, :], psm2[:],
                                            gb_sb[:, 4, dt:dt + 1])

    # ---------------- LN2 + residual + transpose out ----------------
    with tc.tile_pool(name="sqpool2", bufs=3) as sqpool2, \
         tc.tile_pool(name="stpsum2", bufs=2, space="PSUM") as stpsum2, \
         tc.tile_pool(name="stpool2", bufs=2) as stpool2, \
         tc.tile_pool(name="trpsum", bufs=2, space="PSUM") as trpsum, \
         tc.tile_pool(name="outpool", bufs=2) as outpool:
        outT = sqpool2.tile([P, DK, SL], fp32, tag="outT")
        layer_norm_T(mlp_sb, outT, 2, 3, x1t_sb, (sqpool2, stpsum2, stpool2))
        for st in range(NST):
            ob = outpool.tile([SLT, D], fp32, tag="ob")
            for g in range(DK):
                pst = trpsum.tile([SLT, P], fp32, tag="pst")
                nc.tensor.transpose(pst[:], outT[:, g, ts(st, SLT)], ident_sb[:])
                nc.vector.tensor_copy(ob[:, ts(g, P)], pst[:])
            nc.sync.dma_start(io["out"][ts(st, SLT), :], ob[:])
    _ctx.close()


def build_program(S=S_FULL):
    import concourse.mybir as mybir
    import concourse.tile as tile
    from concourse import bacc

    nc = bacc.Bacc("TRN2", target_bir_lowering=False, debug=False,
                   enable_asserts=True, num_devices=NC)
    f32, bf16 = mybir.dt.float32, mybir.dt.bfloat16
    SL = S // NC

    def din(name, shape, dt=bf16):
        return nc.dram_tensor(name, shape, dt, kind="ExternalInput").ap()

    f32r = mybir.dt.float32r
    io = {
        "eT": din("eT", [D, S], f32r),
        "eresT": din("eresT", [D, SL], f32),
        "maskT": din("maskT", [S, S]),
        "wq": din("wq", [D, D], f32r), "wk": din("wk", [D, D], f32r),
        "wv": din("wv", [D, D], f32r),
        "htq": din("htq", [D, EPC], f32r), "htk": din("htk", [D, EPC], f32r),
        "htv": din("htv", [D, EPC], f32r),
        "wot": din("wot", [D, D]),
        "l1t": din("l1t", [D, DM], f32r), "l2t": din("l2t", [DM, D], f32r),
        "l1b": din("l1b", [DM], f32), "l2b": din("l2b", [D], f32),
        "g1": din("g1", [D], f32), "b1": din("b1", [D], f32),
        "g2": din("g2", [D], f32), "b2": din("b2", [D], f32),
        "out": nc.dram_tensor("out", [SL, D], f32, kind="ExternalOutput").ap(),
    }
    with tile.TileContext(nc) as tc:
        _body(tc, io, S)
    nc.compile()
    return nc


def make_in_maps(E, mask, W_Q, W_K, W_V, W_O, H_Q, H_K, H_V,
                 L1_w, L1_b, L2_w, L2_b, gamma1, beta1, gamma2, beta2, S=S_FULL):
    E = np.asarray(E, np.float32).reshape(S, D)
    SL = S // NC
    eT = np.ascontiguousarray(E.T)
    maskT = (np.asarray(mask).T != 0).astype(BF16)
    maskT = np.ascontiguousarray(maskT)
    com = {
        "eT": eT, "maskT": maskT,
        "wq": np.asarray(W_Q, np.float32),
        "wk": np.asarray(W_K, np.float32),
        "wv": np.asarray(W_V, np.float32),
        "wot": np.ascontiguousarray(np.asarray(W_O, np.float32).T).astype(BF16),
        "l1t": np.ascontiguousarray(np.asarray(L1_w, np.float32).T),
        "l2t": np.ascontiguousarray(np.asarray(L2_w, np.float32).T),
        "l1b": np.asarray(L1_b, np.float32), "l2b": np.asarray(L2_b, np.float32),
        "g1": np.asarray(gamma1, np.float32), "b1": np.asarray(beta1, np.float32),
        "g2": np.asarray(gamma2, np.float32), "b2": np.asarray(beta2, np.float32),
    }
    in_maps = []
    for c in range(NC):
        m = dict(com)
        m["eresT"] = np.ascontiguousarray(E[c * SL:(c + 1) * SL, :].T)
        for key, Hw in (("htq", H_Q), ("htk", H_K), ("htv", H_V)):
            hs = np.asarray(Hw, np.float32)[c * HPC:(c + 1) * HPC].reshape(EPC, D)
            m[key] = np.ascontiguousarray(hs.T)
        in_maps.append(m)
    return in_maps


_PROGRAM_CACHE = {}


def kernel(**inputs):
    from concourse import bass_utils
    S = inputs["E"].shape[1]
    if S not in _PROGRAM_CACHE:
        _PROGRAM_CACHE[S] = build_program(S)
    nc = _PROGRAM_CACHE[S]
    in_maps = make_in_maps(S=S, **inputs)
    res = bass_utils.run_bass_kernel_spmd(nc, in_maps, core_ids=list(range(NC)))
    SL = S // NC
    out = np.concatenate([res.results[c]["out"] for c in range(NC)], axis=0)
    return out.reshape(1, S, D).astype(np.float32)


# revision 25
# speedup vs baseline: 181.9083x; 181.9083x over previous
"""Trainium2 Bass kernel for nn_DecoderBlock (B=1, S=2048, D=1024, H=16, DQ=64, DM=4096).

Strategy (8 NeuronCores, one chip):
  - Attention tensor-parallel over heads: core c owns heads {2c, 2c+1}.
    Per-head projections are fused with the QKV projections:
      weff_x[h] = H_x[h] @ W_x  (computed on device), then
      qT/kT = weffT.T-contractions against E.T; v computed in natural [t, e] layout.
    Scores are computed TRANSPOSED ([keys, queries] layout) so that softmax's
    reduction lands on the matmul contraction axis: the row-of-ones trick in the
    o-matmul produces the softmax denominators for free, and no transposes of the
    big score matrix are ever needed.
  - One AllToAll exchanges attention outputs (heads-sharded -> sequence-sharded).
  - Post-attention (W_O, LN1, MLP, LN2, residuals) data-parallel over tokens:
    core c owns tokens [c*256, (c+1)*256), all in transposed [d, s] layout;
    LayerNorm statistics via ones-matmuls over the partition axis.
  - Host-side prep is layout-only: slicing, transposition and dtype tagging of
    inputs; all FLOPs (matmuls, softmax, layernorm) run on device. Matmul chain
    runs in float32r (tf32-class, full PE rate at free-dim>=256, measured
    rms ~1.5e-4); the causal mask is an fp8 0/1 multiplier, the AllToAll payload
    and the MLP's second weight matrix are bf16.
"""

import numpy as np
import ml_dtypes

BF16 = ml_dtypes.bfloat16

B, S_FULL, D, H, DQ, DM = 1, 2048, 1024, 16, 64, 4096
NC = 8          # cores
P = 128         # partitions
HPC = H // NC   # heads per core
EPC = HPC * DQ  # per-core attention width (128)
DK = D // P     # d-model chunks (8)
MK = DM // P    # mlp chunks (32)
EPS = 1e-5


def _body(tc, io, S):
    import concourse.bass as bass
    import concourse.mybir as mybir
    from concourse.masks import make_identity

    nc = tc.nc
    fp32 = mybir.dt.float32
    bf16 = mybir.dt.bfloat16
    f32r = mybir.dt.float32r
    f8 = mybir.dt.float8e4
    Exp = mybir.ActivationFunctionType.Exp
    Relu = mybir.ActivationFunctionType.Relu
    Sqrt = mybir.ActivationFunctionType.Sqrt
    sub_op = mybir.AluOpType.subtract
    mult_op = mybir.AluOpType.mult
    add_op = mybir.AluOpType.add

    SL = S // NC                 # tokens per core
    TT = S // P                  # key tiles
    SPW = min(1024, S)           # score-tile width along queries
    NSP = S // SPW
    W5 = min(512, SPW)           # matmul free-dim chunk
    N5 = SPW // W5
    SLT = min(P, SL)             # partition tile for final transpose
    NST = (SL + P - 1) // P

    ts = bass.ts

    # ---------------- persistent SBUF ----------------
    import contextlib
    _ctx = contextlib.ExitStack()
    persist = _ctx.enter_context(tc.tile_pool(name="persist", bufs=1))

    def ptile(shape, dt, tag):
        return persist.tile(shape, dt, tag=tag, name=tag)

    drampool = _ctx.enter_context(tc.tile_pool(name="drampool", bufs=1, space="DRAM"))

    qt_sb = ptile([EPC, S], f32r, "qt_sb")
    kt_sb = ptile([EPC, S], f32r, "kt_sb")
    ot_sb = ptile([EPC, S], bf16, "ot_sb")            # attn out (2 heads)
    weff_sb = ptile([P, 3, DK, P], f32r, "weff_sb")   # weffT q/k/v
    vaug_sb = ptile([P, HPC, TT, DQ + 1], f32r, "vaug_sb")
    ones_sb = ptile([P, 1], fp32, "ones_sb")
    ident_sb = ptile([P, P], fp32, "ident_sb")
    onesr_sb = ptile([P, 1], f32r, "onesr_sb")
    gb_sb = ptile([P, 6, DK], fp32, "gb_sb")          # g1,b1,g2,b2,l2b,(pad)
    l1b_sb = ptile([P, MK], fp32, "l1b_sb")
    x1t_sb = ptile([P, DK, SL], f32r, "x1t_sb")
    att_sb = ptile([P, DK, SL], f32r, "att_sb")
    mlp_sb = ptile([P, DK, SL], f32r, "mlp_sb")
    ots_sb = ptile([P, DK, SL], bf16, "ots_sb")       # o for my tokens
    wot_sb = ptile([P, DK, D], bf16, "wot_sb")

    nc.gpsimd.memset(ones_sb[:], 1.0)
    nc.vector.tensor_copy(onesr_sb[:], ones_sb[:])
    make_identity(nc, ident_sb[:])
    for i, name in enumerate(("g1", "b1", "g2", "b2", "l2b")):
        nc.sync.dma_start(gb_sb[:, i, :], io[name].rearrange("(g p) -> p g", p=P))
    nc.sync.dma_start(l1b_sb[:], io["l1b"].rearrange("(g p) -> p g", p=P))

    # ---------------- weffT = W.T @ H.T ----------------
    etctx = contextlib.ExitStack()
    etpool = etctx.enter_context(tc.tile_pool(name="etpool", bufs=1))
    et_sb = etpool.tile([P, DK, S], f32r, tag="et_sb", name="et_sb")
    for g in range(DK):
        nc.sync.dma_start(et_sb[:, g, :], io["eT"][ts(g, P), :])

    identr_sb = ptile([P, P], f32r, "identr_sb")
    nc.vector.tensor_copy(identr_sb[:], ident_sb[:])
    with tc.tile_pool(name="wpool", bufs=2) as wpool, \
         tc.tile_pool(name="wepool", bufs=2) as wepool, \
         tc.tile_pool(name="wpsum", bufs=2, space="PSUM") as wpsum, \
         tc.tile_pool(name="wtpsum", bufs=2, space="PSUM") as wtpsum:
        for j, (wn, hn) in enumerate((("wq", "htq"), ("wk", "htk"), ("wv", "htv"))):
            ht = wpool.tile([P, DK, EPC], f32r, tag="ht")
            for kk in range(DK):
                nc.sync.dma_start(ht[:, kk, :], io[hn][ts(kk, P), :])
            # weff_ed[e, din] = sum_dout H.T[dout, e].T @ W[dout, din]
            wed = wepool.tile([EPC, D], f32r, tag="wed")
            for dh in range(D // 512):
                wt = wpool.tile([P, DK, 512], f32r, tag="wt")
                nc.sync.dma_start(
                    wt[:], io[wn][:, ts(dh, 512)].rearrange("(kk p) e -> p kk e", p=P))
                ps = wpsum.tile([EPC, 512], fp32, tag="wps")
                for kk in range(DK):
                    nc.tensor.matmul(ps[:], ht[:, kk, :], wt[:, kk, :],
                                     start=(kk == 0), stop=(kk == DK - 1))
                if j == 0:  # fold 1/sqrt(DQ) into q weights
                    nc.vector.tensor_scalar_mul(wed[:, ts(dh, 512)], ps[:], 1.0 / np.sqrt(DQ))
                else:
                    nc.vector.tensor_copy(wed[:, ts(dh, 512)], ps[:])
            for m in range(DK):
                pst = wtpsum.tile([P, EPC], f32r, tag="wtps")
                nc.tensor.transpose(pst[:], wed[:, ts(m, P)], identr_sb[:])
                nc.vector.tensor_copy(weff_sb[:, j, m, :], pst[:])

    # ---------------- qT, kT, v ----------------
    with tc.tile_pool(name="qkpsum", bufs=3, space="PSUM") as qkpsum:
        for j, dst in ((0, qt_sb), (1, kt_sb)):
            for sp in range(S // 512):
                ps = qkpsum.tile([EPC, 512], fp32, tag="qkps")
                for kk in range(DK):
                    nc.tensor.matmul(ps[:], weff_sb[:, j, kk, :], et_sb[:, kk, ts(sp, 512)],
                                     start=(kk == 0), stop=(kk == DK - 1))
                nc.vector.tensor_copy(dst[:, ts(sp, 512)], ps[:])
        with tc.tile_pool(name="vtpool", bufs=1) as vtpool, \
             tc.tile_pool(name="vtpsum", bufs=2, space="PSUM") as vtpsum:
            vt = vtpool.tile([EPC, S], f32r, tag="vt")
            for sp in range(S // 512):
                ps = qkpsum.tile([EPC, 512], fp32, tag="qkps")
                for kk in range(DK):
                    nc.tensor.matmul(ps[:], weff_sb[:, 2, kk, :], et_sb[:, kk, ts(sp, 512)],
                                     start=(kk == 0), stop=(kk == DK - 1))
                nc.vector.tensor_copy(vt[:, ts(sp, 512)], ps[:])
            for t in range(TT):
                pst = vtpsum.tile([P, EPC], f32r, tag="vtps")
                nc.tensor.transpose(pst[:], vt[:, ts(t, P)], identr_sb[:])
                for h in range(HPC):
                    nc.vector.tensor_copy(vaug_sb[:, h, t, :DQ], pst[ts(h, DQ), :].rearrange("e t -> t e") if False else pst[:, ts(h, DQ)])
                    nc.vector.tensor_copy(vaug_sb[:, h, t, DQ:], onesr_sb[:])
    etctx.close()
    if stop_after == "qkv":
        _ctx.close(); return

    # ---------------- attention ----------------
    with tc.tile_pool(name="mkpool", bufs=3) as mkpool, \
         tc.tile_pool(name="expool", bufs=3) as expool, \
         tc.tile_pool(name="scpsum", bufs=2, space="PSUM") as scpsum, \
         tc.tile_pool(name="opsum", bufs=1, space="PSUM") as opsum, \
         tc.tile_pool(name="nrmpool", bufs=2) as nrmpool:
        for sp in range(NSP):
            pso = [opsum.tile([DQ + 1, SPW], fp32, tag=f"pso{h}", name=f"pso{h}")
                   for h in range(HPC)]
            for t in range(TT):
                mk = mkpool.tile([P, SPW], f8, tag="mk")
                nc.sync.dma_start(mk[:], io["maskT"][ts(t, P), ts(sp, SPW)])
                for h in range(HPC):
                    ps = scpsum.tile([P, SPW], fp32, tag="scps")
                    hs = slice(h * DQ, (h + 1) * DQ)
                    for n in range(N5):
                        nc.tensor.matmul(
                            ps[:, ts(n, W5)], kt_sb[hs, ts(t, P)],
                            qt_sb[hs, sp * SPW + n * W5:sp * SPW + (n + 1) * W5],
                            start=True, stop=True)
                    ex = expool.tile([P, SPW], f32r, tag="ex")
                    nc.scalar.activation(ex[:], ps[:], Exp)
                    nc.vector.tensor_mul(ex[:], ex[:], mk[:])
                    for n in range(N5):
                        nc.tensor.matmul(pso[h][:, ts(n, W5)], vaug_sb[:, h, t, :],
                                         ex[:, ts(n, W5)],
                                         start=(t == 0), stop=(t == TT - 1))
            for h in range(HPC):
                rc = nrmpool.tile([1, SPW], fp32, tag="rc")
                nc.vector.reciprocal(rc[:], pso[h][DQ:DQ + 1, :])
                rcb = nrmpool.tile([DQ, SPW], fp32, tag="rcb")
                nc.gpsimd.partition_broadcast(rcb[:], rc[:], channels=DQ)
                nc.vector.tensor_mul(ot_sb[ts(h, DQ), ts(sp, SPW)], pso[h][:DQ, :],
                                     rcb[:])

    if stop_after == "attn":
        _ctx.close(); return

    # ---------------- AllToAll: heads-sharded -> token-sharded ----------------
    a2a_in = drampool.tile([NC * EPC, SL], bf16, tag="a2a_in", name="a2a_in")
    a2a_out = drampool.tile([NC * EPC, SL], bf16, tag="a2a_out", name="a2a_out")
    for g in range(NC):
        nc.sync.dma_start(a2a_in[ts(g, EPC), :], ot_sb[:, ts(g, SL)])
    nc.gpsimd.collective_compute(
        "AllToAll", mybir.AluOpType.bypass,
        replica_groups=[list(range(NC))],
        ins=[a2a_in.opt()], outs=[a2a_out.opt()])
    for g in range(NC):
        nc.sync.dma_start(ots_sb[:, g, :], a2a_out[ts(g, EPC), :])

    # ---------------- W_O projection (attT) ----------------
    for g in range(DK):
        nc.sync.dma_start(wot_sb[:, g, :], io["wot"][ts(g, P), :])
    with tc.tile_pool(name="atpsum", bufs=2, space="PSUM") as atpsum:
        for m in range(DK):
            ps = atpsum.tile([P, SL], fp32, tag="atps")
            for g in range(DK):
                nc.tensor.matmul(ps[:], wot_sb[:, g, ts(m, P)], ots_sb[:, g, :],
                                 start=(g == 0), stop=(g == DK - 1))
            nc.vector.tensor_copy(att_sb[:, m, :], ps[:])

    if stop_after == "wo":
        _ctx.close(); return

    # ---------------- LayerNorm helper (stats over partition axis) -------------
    def layer_norm_T(src_sb, dst_sb, gi, bi, res_sb, pools):
        """dst = res + gamma*(src-mean)/sqrt(var+eps) + beta, all [P, DK, SL] f32."""
        sqpool, stpsum, stpool = pools
        ps1 = stpsum.tile([1, SL], fp32, tag="ps1")
        ps2 = stpsum.tile([1, SL], fp32, tag="ps2")
        for g in range(DK):
            sq = sqpool.tile([P, SL], f32r, tag="sq")
            nc.vector.tensor_mul(sq[:], src_sb[:, g, :], src_sb[:, g, :])
            nc.tensor.matmul(ps1[:], onesr_sb[:], src_sb[:, g, :],
                             start=(g == 0), stop=(g == DK - 1))
            nc.tensor.matmul(ps2[:], onesr_sb[:], sq[:],
                             start=(g == 0), stop=(g == DK - 1))
        mean = stpool.tile([1, SL], fp32, tag="mean")
        nc.vector.tensor_scalar_mul(mean[:], ps1[:], 1.0 / D)
        var = stpool.tile([1, SL], fp32, tag="var")
        nc.vector.tensor_scalar_mul(var[:], ps2[:], 1.0 / D)
        msq = stpool.tile([1, SL], fp32, tag="msq")
        nc.vector.tensor_mul(msq[:], mean[:], mean[:])
        nc.vector.tensor_tensor(var[:], var[:], msq[:], sub_op)
        nc.vector.tensor_scalar_add(var[:], var[:], EPS)
        std = stpool.tile([1, SL], fp32, tag="std")
        nc.scalar.activation(std[:], var[:], Sqrt)
        rstd = stpool.tile([1, SL], fp32, tag="rstd")
        nc.vector.reciprocal(rstd[:], std[:])
        mrs = stpool.tile([1, SL], fp32, tag="mrs")  # mean*rstd
        nc.vector.tensor_mul(mrs[:], mean[:], rstd[:])
        rstdb = stpool.tile([P, SL], fp32, tag="rstdb")
        nc.gpsimd.partition_broadcast(rstdb[:], rstd[:])
        mrsb = stpool.tile([P, SL], fp32, tag="mrsb")
        nc.gpsimd.partition_broadcast(mrsb[:], mrs[:])
        for g in range(DK):
            t1 = sqpool.tile([P, SL], f32r, tag="t1")
            nc.vector.tensor_mul(t1[:], src_sb[:, g, :], rstdb[:])
            nc.vector.tensor_tensor(t1[:], t1[:], mrsb[:], sub_op)
            nc.vector.tensor_scalar(t1[:], t1[:], gb_sb[:, gi, g:g + 1],
                                    gb_sb[:, bi, g:g + 1], mult_op, add_op)
            nc.vector.tensor_tensor(dst_sb[:, g, :], t1[:], res_sb[:, g, :], add_op)

    with tc.tile_pool(name="sqpool", bufs=3) as sqpool, \
         tc.tile_pool(name="stpsum", bufs=2, space="PSUM") as stpsum, \
         tc.tile_pool(name="stpool", bufs=2) as stpool:
        eres = sqpool.tile([P, DK, SL], fp32, tag="eres")
        for g in range(DK):
            nc.sync.dma_start(eres[:, g, :], io["eresT"][ts(g, P), :])
        layer_norm_T(att_sb, x1t_sb, 0, 1, eres, (sqpool, stpsum, stpool))

    if stop_after == "ln1":
        _ctx.close(); return

    # ---------------- MLP ----------------
    with tc.tile_pool(name="l1pool", bufs=4) as l1pool, \
         tc.tile_pool(name="hallpool", bufs=1) as hallpool, \
         tc.tile_pool(name="hpsum", bufs=3, space="PSUM") as hpsum:
        ht_all = hallpool.tile([P, MK, SL], bf16, tag="ht_all")
        for mg in range(MK // 4):  # m-chunk groups of 4
            l1t = l1pool.tile([P, DK, 512], f32r, tag="l1t")
            for g in range(DK):
                nc.sync.dma_start(l1t[:, g, :], io["l1t"][ts(g, P), ts(mg, 512)])
            for mi in range(4):
                mc = mg * 4 + mi
                psh = hpsum.tile([P, SL], fp32, tag="psh")
                for g in range(DK):
                    nc.tensor.matmul(psh[:], l1t[:, g, ts(mi, P)], x1t_sb[:, g, :],
                                     start=(g == 0), stop=(g == DK - 1))
                nc.scalar.activation(ht_all[:, mc, :], psh[:], Relu,
                                     bias=l1b_sb[:, mc:mc + 1])
        with tc.tile_pool(name="l2pool", bufs=3) as l2pool, \
             tc.tile_pool(name="mlppsum", bufs=2, space="PSUM") as mlppsum:
            for dt in range(DK):
                l2td = l2pool.tile([P, MK, P], bf16, tag="l2td")
                nc.sync.dma_start(l2td[:], io["l2t"][dt])
                psm2 = mlppsum.tile([P, SL], fp32, tag="psm2")
                for mc in range(MK):
                    nc.tensor.matmul(psm2[:], l2td[:, mc, :], ht_all[:, mc, :],
                                     start=(mc == 0), stop=(mc == MK - 1))
                nc.vector.tensor_scalar_add(mlp_sb[:, dt, :], psm2[:],
                                            gb_sb[:, 4, dt:dt + 1])

    if stop_after == "mlp":
        _ctx.close(); return

    # ---------------- LN2 + residual + transpose out ----------------
    with tc.tile_pool(name="sqpool2", bufs=3) as sqpool2, \
         tc.tile_pool(name="stpsum2", bufs=2, space="PSUM") as stpsum2, \
         tc.tile_pool(name="stpool2", bufs=2) as stpool2, \
         tc.tile_pool(name="trpsum", bufs=2, space="PSUM") as trpsum, \
         tc.tile_pool(name="outpool", bufs=2) as outpool:
        outT = sqpool2.tile([P, DK, SL], f32r, tag="outT")
        layer_norm_T(mlp_sb, outT, 2, 3, x1t_sb, (sqpool2, stpsum2, stpool2))
        for st in range(NST):
            ob = outpool.tile([SLT, D], fp32, tag="ob")
            for g in range(DK):
                pst = trpsum.tile([SLT, P], f32r, tag="pst")
                nc.tensor.transpose(pst[:], outT[:, g, ts(st, SLT)], identr_sb[:])
                nc.vector.tensor_copy(ob[:, ts(g, P)], pst[:])
            nc.sync.dma_start(io["out"][ts(st, SLT), :], ob[:])
    _ctx.close()


def build_program(S=S_FULL):
    import concourse.mybir as mybir
    import concourse.tile as tile
    from concourse import bacc

    nc = bacc.Bacc("TRN2", target_bir_lowering=False, debug=False,
                   enable_asserts=True, num_devices=NC)
    f32, bf16 = mybir.dt.float32, mybir.dt.bfloat16
    SL = S // NC

    def din(name, shape, dt=bf16):
        return nc.dram_tensor(name, shape, dt, kind="ExternalInput").ap()

    f32r = mybir.dt.float32r
    io = {
        "eT": din("eT", [D, S], f32r),
        "eresT": din("eresT", [D, SL], f32),
        "maskT": din("maskT", [S, S], mybir.dt.float8e4),
        "wq": din("wq", [D, D], f32r), "wk": din("wk", [D, D], f32r),
        "wv": din("wv", [D, D], f32r),
        "htq": din("htq", [D, EPC], f32r), "htk": din("htk", [D, EPC], f32r),
        "htv": din("htv", [D, EPC], f32r),
        "wot": din("wot", [D, D]),
        "l1t": din("l1t", [D, DM], f32r), "l2t": din("l2t", [DK, P, MK, P]),
        "l1b": din("l1b", [DM], f32), "l2b": din("l2b", [D], f32),
        "g1": din("g1", [D], f32), "b1": din("b1", [D], f32),
        "g2": din("g2", [D], f32), "b2": din("b2", [D], f32),
        "out": nc.dram_tensor("out", [SL, D], f32, kind="ExternalOutput").ap(),
    }
    with tile.TileContext(nc) as tc:
        _body(tc, io, S)
    nc.compile()
    return nc


def make_in_maps(E, mask, W_Q, W_K, W_V, W_O, H_Q, H_K, H_V,
                 L1_w, L1_b, L2_w, L2_b, gamma1, beta1, gamma2, beta2, S=S_FULL):
    E = np.asarray(E, np.float32).reshape(S, D)
    SL = S // NC
    eT = np.ascontiguousarray(E.T)
    import ml_dtypes as _mld
    maskT = (np.asarray(mask).T != 0).astype(_mld.float8_e4m3)
    maskT = np.ascontiguousarray(maskT)
    com = {
        "eT": eT, "maskT": maskT,
        "wq": np.asarray(W_Q, np.float32),
        "wk": np.asarray(W_K, np.float32),
        "wv": np.asarray(W_V, np.float32),
        "wot": np.ascontiguousarray(np.asarray(W_O, np.float32).T).astype(BF16),
        "l1t": np.ascontiguousarray(np.asarray(L1_w, np.float32).T),
        "l2t": np.ascontiguousarray(
            np.asarray(L2_w, np.float32).T.reshape(MK, P, DK, P).transpose(2, 1, 0, 3)
        ).astype(BF16),
        "l1b": np.asarray(L1_b, np.float32), "l2b": np.asarray(L2_b, np.float32),
        "g1": np.asarray(gamma1, np.float32), "b1": np.asarray(beta1, np.float32),
        "g2": np.asarray(gamma2, np.float32), "b2": np.asarray(beta2, np.float32),
    }
    in_maps = []
    for c in range(NC):
        m = dict(com)
        m["eresT"] = np.ascontiguousarray(E[c * SL:(c + 1) * SL, :].T)
        for key, Hw in (("htq", H_Q), ("htk", H_K), ("htv", H_V)):
            hs = np.asarray(Hw, np.float32)[c * HPC:(c + 1) * HPC].reshape(EPC, D)
            m[key] = np.ascontiguousarray(hs.T)
        in_maps.append(m)
    return in_maps


_PROGRAM_CACHE = {}


def kernel(**inputs):
    from concourse import bass_utils
    S = inputs["E"].shape[1]
    if S not in _PROGRAM_CACHE:
        _PROGRAM_CACHE[S] = build_program(S)
    nc = _PROGRAM_CACHE[S]
    in_maps = make_in_maps(S=S, **inputs)
    res = bass_utils.run_bass_kernel_spmd(nc, in_maps, core_ids=list(range(NC)))
    SL = S // NC
    out = np.concatenate([res.results[c]["out"] for c in range(NC)], axis=0)
    return out.reshape(1, S, D).astype(np.float32)
